# revision 31
# baseline (speedup 1.0000x reference)
"""Trainium2 Bass kernel for nn_EyeRobotAgent block-sparse ("eye") attention.

Shapes: q,k,v [2, 12, 3456, 32] fp32.  S = 16 time-blocks x 216 feats.
Mask structure (per query block t):
  - img queries (m in [20,216), 196 of them) see only the "core" keys:
    19 keys (m in {0..3,5..19}) of each block tau in [t-7, t] plus m4(t)
    -> at most 153 keys,
  - non-img queries (m in [0,20), 20 of them) see core keys + the 196 img
    keys of block t (joint queries additionally lose past joint keys,
    handled by a bias row).

Strategy (data-parallel: 24 (b,h) pairs over 8 cores, 3 each):
  Per block pack keys as [core 153 | img 196 | pad] = 384 (newest-first so
  invalid tail cols are contiguous; masks fold into 2 bias contraction
  rows).  Scores are computed transposed [kv, q] in per-quad (4 block)
  PSUM tiles so a single ACT exp covers ~1000-1250 columns:
    N_j: 20 non-img queries vs 2-3 full-height 128-row chunks of the pack
    A_j: 196 img queries vs core[0:128]
    C:   core[128:153] tails of the 4 blocks packed into 32-row PE
         quadrant bands (tile_position rows 32j), one shared 196-col region
    PAD: 28 dummy cols kept defined so PV lhsT "spill" reads stay legal.
  PV uses probs as the stationary operand (out[q, 33] per matmul streams
  only 33 columns); the appended ones-column of V yields the softmax
  denominator in col 32; normalize is one DVE reciprocal + one mul per
  quad, padded out-groups making every partition defined.
  exp() has no max-subtraction (scores are O(6), fp32-safe).
"""
import math
import numpy as np

import concourse.bass as bass
import concourse.mybir as mybir
import concourse.tile as tile
from concourse import bacc
from concourse.bass_utils import run_bass_kernel_spmd
from concourse.tile_rust import add_dep_helper

B, H, S, D = 2, 12, 3456, 32
F = 216            # feats_per_t
W = 8              # window_len
T = S // F         # 16 blocks
IMG_START = 20     # F - img_feat_size
PAST_SEL = np.array([0, 1, 2, 3] + list(range(5, 20)))   # 19 per block
NCORE = 153        # 8*19 + 1 (m4) candidate core keys
NIMG = 196
PACK = 384         # [core (<=153, compact) | img 196 | pad]
KAUG = D + 3       # 35 = 32 d + joint-bias + img-img bias + validity rows
VA = D + 1         # 33 = v columns + ones column
NEG = np.float32(-30000.0)
SCALE = float(1.0 / np.sqrt(np.float32(D)))
N_CORES = 8
BH_PER_CORE = (B * H) // N_CORES      # 3
NQ = 4                                # blocks per quad
QUADS = T // NQ                       # 4

F32 = mybir.dt.float32
BF16 = mybir.dt.float16      # half precision: matmul rate 1 cyc/row
NP_BF16 = np.float16


def _nvalid_core(t):
    return 20 + 19 * min(t, 7)


def _n_chunks(t):
    return math.ceil((_nvalid_core(t) + NIMG) / 128)


def _pack_rows(t):
    """Compact key packing for block t: [19(t), m4(t), 19(t-1), ..,
    19(t-min(t,7)), img(t) 196, pad].  -1 marks invalid (trailing only)."""
    rows = list(F * t + PAST_SEL) + [F * t + 4]
    for s in range(1, min(t, 7) + 1):
        rows += list(F * (t - s) + PAST_SEL)
    rows += list(range(F * t + IMG_START, F * t + F))
    rows += [-1] * (PACK - len(rows))
    return np.array(rows)


def _quad_layout(g):
    """Column layout of the per-quad score tile, bank-aware (matmul outputs
    must not cross 512-col PSUM bank boundaries): bank0 = [A0 A1 N x6],
    bank1 = [A2 A3 N-rest pad?], bank2 = [C... pad].  C regions are per
    block PAIR (bands rows 0:32 / 32:64 by t%2, rows 64:128 dummy-filled).
    Exp'd cols [0:ncols] are gap-free; 60 pad cols keep PV spill reads
    defined."""
    ts = list(range(NQ * g, NQ * g + NQ))
    chunks = [(j, c) for j, t in enumerate(ts) for c in range(_n_chunks(t))]
    a_off = [0, 196, 512, 708]
    n_cols = {}
    col = 392
    for jc in chunks[:6]:
        n_cols[jc] = col
        col += 20
    assert col <= 512
    col = 904
    for jc in chunks[6:]:
        n_cols[jc] = col
        col += 20
    pads = []
    if col < 1024:
        pads.append((col, 1024 - col))
    c_off = {}
    cbase = 1024
    for pl in range(NQ // 2):           # local pair index
        if any(_nvalid_core(t) > 128 for t in ts[2 * pl:2 * pl + 2]):
            c_off[pl] = cbase
            cbase += NIMG
    pads.append((cbase, 60))
    return ts, n_cols, a_off, c_off, pads, cbase + 60


# ---------------------------------------------------------------- host packing

def _pack_all(q, k, v):
    nbh = B * H
    qf = q.reshape(nbh, S, D)
    kf = k.reshape(nbh, S, D)
    vf = v.reshape(nbh, S, D)
    qm = np.arange(S) % F

    # qt/kpt live in two partition bands (rows 0:35 = blocks 0..7, rows
    # 64:99 = blocks 8..15): v1-model DMA cost is per-partition bytes, so
    # spreading over 2x partitions halves the transfer time.
    qtf = np.zeros((nbh, KAUG, S), np.float32)
    qtf[:, :D] = qf.transpose(0, 2, 1)
    qtf[:, 32] = (qm >= 4) & (qm < IMG_START)      # is_joint(q)
    qtf[:, 33] = qm >= IMG_START                   # is_img(q)
    qtf[:, 34] = 1.0
    qt = np.zeros((nbh, 99, S // 2), np.float32)
    qt[:, 0:KAUG] = qtf[:, :, 0:S // 2]
    qt[:, 64:64 + KAUG] = qtf[:, :, S // 2:]

    kpt = np.zeros((nbh, 99, T // 2, PACK), np.float32)
    vp = np.zeros((nbh, 128, T, 3, VA), np.float32)
    vpc = np.zeros((nbh, 128, T // 2, VA), np.float32)
    for t in range(T):
        rows = _pack_rows(t)
        valid = rows >= 0
        safe = np.where(valid, rows, 0)
        kb, tl = 64 * (t // 8), t % 8
        kpt[:, kb:kb + D, tl] = np.where(
            valid[None, None], kf[:, safe].transpose(0, 2, 1), 0.0)
        # joint-past bias: past sets s=1..min(t,7) at cols 20+19(s-1),
        # joint keys at positions 4..18 within each set
        jbias = np.zeros(PACK, np.float32)
        for s in range(1, min(t, 7) + 1):
            base = 20 + 19 * (s - 1)
            jbias[base + 4: base + 19] = NEG
        kpt[:, kb + 32, tl] = jbias
        kpt[:, kb + 33, tl] = np.where(valid & (rows % F >= IMG_START), NEG, 0.0)
        kpt[:, kb + 34, tl] = np.where(valid, 0.0, NEG)
        vblk = np.where(valid[None, :, None], vf[:, safe], 0.0)   # [nbh,384,32]
        vp[:, :, t, :, :D] = vblk.reshape(nbh, 3, 128, D).transpose(0, 2, 1, 3)
        # ones column: eps (not 0) on invalid rows keeps every PV spill-row
        # denominator strictly positive (invalid probs are exactly 0, so
        # valid outputs are unaffected).
        ones = np.where(valid, 1.0, 6e-5)
        vp[:, :, t, :, 32] = ones.reshape(3, 128).T[None]
        pr, b = t // 2, 32 * (t % 2)
        # C tail: only core positions 128..nvalid_core-1 (img keys that
        # fall in [128:153] of the compact pack must contribute zero)
        ncv = _nvalid_core(t)
        tail = safe[128:NCORE]
        tval = np.arange(128, NCORE) < ncv
        vpc[:, b:b + 25, pr, :D] = np.where(tval[None, :, None], vf[:, tail], 0.0)
        vpc[:, b:b + 25, pr, 32] = np.where(tval, 1.0, 0.0)
    # fold vpc after vp in one tensor (one DMA)
    vcomb = np.concatenate(
        [vp.reshape(nbh, 128, T * 3 * VA), vpc.reshape(nbh, 128, T // 2 * VA)],
        axis=2)
    return {"qt": np.ascontiguousarray(qt.astype(NP_BF16)),
            "kpt": np.ascontiguousarray(kpt.astype(NP_BF16)),
            "vp": np.ascontiguousarray(vcomb.astype(NP_BF16))}


def _unpack(arr):
    """arr [n, 128, QUADS*320] staging -> [n, S, D].  Per quad g, 10 groups
    of 32 cols: j=0..3 img q 20..147 (rows 0:128); 4..7 img q 148..215
    (rows 0:68); 8..9 non-img q 0..19 of blocks 2p (rows 0:20) and 2p+1
    (rows 64:84)."""
    n = arr.shape[0]
    arr = arr.astype(np.float32)
    r = arr.reshape(n, 128, QUADS, 10, 32).transpose(0, 2, 3, 1, 4)
    out = np.empty((n, QUADS, NQ, F, D), np.float32)
    for j in range(NQ):
        out[:, :, j, IMG_START:148] = r[:, :, j, 0:128]
        out[:, :, j, 148:] = r[:, :, 4 + j, 0:68]
        out[:, :, j, :IMG_START] = r[:, :, 8 + j // 2, 64 * (j % 2):64 * (j % 2) + 20]
    return out.reshape(n, S, D)


# ---------------------------------------------------------------- bass kernel

def build_nc(n_bh=BH_PER_CORE):
    nc = bacc.Bacc(None, target_bir_lowering=False, debug=False)
    qt_d = nc.declare_dram_parameter("qt", [n_bh, 99, S // 2], BF16, isOutput=False)
    kpt_d = nc.declare_dram_parameter("kpt", [n_bh, 99, T // 2, PACK], BF16, isOutput=False)
    vp_d = nc.declare_dram_parameter("vp", [n_bh, 128, (T * 3 + T // 2) * VA], BF16, isOutput=False)
    # out mirrors the SBUF staging tile exactly (fp16); host unpacks.
    out_d = nc.declare_dram_parameter("out", [n_bh, 128, QUADS * 320], BF16, isOutput=True)

    def _strided2(ap, d1, d2):
        return bass.AP(tensor=ap.tensor, offset=ap.offset,
                       ap=[list(ap.ap[0]), list(d1), list(d2)])

    with tile.TileContext(nc) as tc:
        with (
            tc.tile_pool(name="singles", bufs=1) as singles,
            tc.tile_pool(name="qtp", bufs=3) as qtp,
            tc.tile_pool(name="kptp", bufs=3) as kptp,
            tc.tile_pool(name="vpp", bufs=3) as vpp,
            tc.tile_pool(name="probsp", bufs=3) as probsp,
            tc.tile_pool(name="recipsp", bufs=3) as recipsp,
            tc.tile_pool(name="outsbp", bufs=3) as outsbp,
            tc.tile_pool(name="scoresp", bufs=2, space="PSUM") as scoresp,
            tc.tile_pool(name="pvp", bufs=2, space="PSUM") as pvp,
        ):
            zero = singles.tile([1, 128], BF16)
            nc.vector.memset(zero[:], 0.0)

            pending_outs = []
            for i in range(n_bh):
                qt_sb = qtp.tile([99, S // 2], BF16)
                kpt_sb = kptp.tile([99, T // 2, PACK], BF16)
                vp_sb = vpp.tile([128, (T * 3 + T // 2) * VA], BF16)
                # halves so quad 0 can start after ~half the input latency;
                # kpt.h1 goes on the Pool queue, parallel with qt.h1 on SP.
                # Prior-bh out-DMAs are emitted AFTER this bh's inputs so
                # they never head-of-line-block the input stream.
                VH = T * 3 * VA // 2
                if i == 0:
                    # split the critical first kpt piece across the (still
                    # idle) ACT and Pool DMA queues: quad 0 is ready ~500ns
                    # sooner, before the ACT exp stream begins
                    nc.scalar.dma_start(out=kpt_sb[:, 0:T // 8, :],
                                        in_=kpt_d[i, :, 0:T // 8, :])
                    nc.gpsimd.dma_start(
                        out=kpt_sb[:, T // 8:T // 4, :],
                        in_=kpt_d[i, :, T // 8:T // 4, :])
                else:
                    nc.gpsimd.dma_start(out=kpt_sb[:, 0:T // 4, :],
                                        in_=kpt_d[i, :, 0:T // 4, :])
                nc.sync.dma_start(out=qt_sb[:, 0:S // 4],
                                  in_=qt_d[i, :, 0:S // 4])
                nc.sync.dma_start(out=qt_sb[:, S // 4:],
                                  in_=qt_d[i, :, S // 4:])
                nc.sync.dma_start(out=kpt_sb[:, T // 4:, :],
                                  in_=kpt_d[i, :, T // 4:, :])
                nc.gpsimd.dma_start(out=vp_sb[:, 0:VH], in_=vp_d[i, :, 0:VH])
                nc.gpsimd.dma_start(out=vp_sb[:, VH:], in_=vp_d[i, :, VH:])
                for fn in pending_outs:
                    fn()
                pending_outs = []
                outst = outsbp.tile([128, QUADS * 320], BF16)

                VPC = T * 3 * VA          # vpc offset inside vp_sb

                def kslice(t, lo, hi):
                    return kpt_sb[64 * (t // 8):64 * (t // 8) + KAUG,
                                  t % 8, lo:hi]

                def qslice(t, lo, hi):
                    base = F * t - (S // 2) * (t // 8)
                    return qt_sb[64 * (t // 8):64 * (t // 8) + KAUG,
                                 base + lo:base + hi]

                def vslice(t, c):
                    return vp_sb[:, (3 * t + c) * VA:(3 * t + c + 1) * VA]

                for g in range(QUADS):
                    ts, n_cols, a_off, c_off, pads, ncols = _quad_layout(g)

                    # ---- QK^T (transposed scores [kv, q]); masks via the 2
                    # bias rows; all mms chained (same-bank group ordering).
                    scores = scoresp.tile([128, 1536], F32)
                    prev = None

                    def mm(out, lhsT, rhs, start=True, stop=True):
                        nonlocal prev
                        m = nc.tensor.matmul(out, lhsT=lhsT, rhs=rhs,
                                             start=start, stop=stop)
                        if prev is not None:
                            add_dep_helper(m.ins, prev.ins, sync=False,
                                           reason="bank group order")
                        prev = m
                        return m

                    for j, t in enumerate(ts):
                        for c in range(_n_chunks(t)):
                            nco = n_cols[(j, c)]
                            mm(scores[0:128, nco:nco + 20],
                               kslice(t, 128 * c, 128 * c + 128),
                               qslice(t, 0, IMG_START))
                    for j, t in enumerate(ts):
                        mm(scores[0:128, a_off[j]:a_off[j] + NIMG],
                           kslice(t, 0, 128),
                           qslice(t, IMG_START, F))
                    for pl, co in c_off.items():
                        for t in ts[2 * pl:2 * pl + 2]:
                            b = 32 * (t % 2)
                            if _nvalid_core(t) > 128:
                                mm(scores[b:b + 32, co:co + NIMG],
                                   kslice(t, 128, 160),
                                   qslice(t, IMG_START, F))
                            else:      # dummy fill: zero scores -> probs 1
                                mm(scores[b:b + 32, co:co + NIMG],
                                   zero[0:1, 0:32], qt_sb[0:1, 0:NIMG])
                        mm(scores[64:128, co:co + NIMG],
                           zero[0:1, 0:64], qt_sb[0:1, 0:NIMG])


                    # ---- probs = exp(scale * scores), one ACT op per
                    # quad; trailing pad cols (PV spill targets) don't need
                    # exp -- Pool memsets them to 1.0 directly.
                    probs = probsp.tile([128, 1536], BF16)
                    pad0 = pads[0][0]
                    nc.scalar.activation(probs[0:128, 0:pad0],
                                         scores[0:128, 0:pad0],
                                         mybir.ActivationFunctionType.Exp,
                                         scale=SCALE)
                    nc.gpsimd.memset(probs[0:128, pad0:ncols], 1.0)

                    # ---- PV: probs stationary, out[q, va] (33-col streams).
                    # 10 out groups: col 33j img q 0..127; col 33(4+j) img q
                    # 128..195 (68 valid + 60 spill); col 33(8+p) non-img of
                    # blocks 2p (rows 0:64) / 2p+1 (rows 64:128), 20 valid
                    # + 44 spill each.  Spill keeps all partitions defined
                    # so one recip + one mul normalizes everything.
                    pv = pvp.tile([128, 10 * VA], F32)
                    prev = None
                    for j, t in enumerate(ts):
                        has_c = _nvalid_core(t) > 128
                        co = c_off.get(j // 2)
                        b = 32 * (t % 2)
                        pr = t // 2
                        mm(pv[0:128, 33 * j:33 * j + VA],
                           probs[0:128, a_off[j]:a_off[j] + 128],
                           vslice(t, 0), start=True, stop=not has_c)
                        if has_c:
                            mm(pv[0:128, 33 * j:33 * j + VA],
                               probs[b:b + 25, co:co + 128],
                               vp_sb[b:b + 25, VPC + pr * VA:VPC + (pr + 1) * VA],
                               start=False, stop=True)
                        mm(pv[0:128, 33 * (4 + j):33 * (4 + j) + VA],
                           probs[0:128, a_off[j] + 128:a_off[j] + 256],
                           vslice(t, 0), start=True, stop=not has_c)
                        if has_c:
                            mm(pv[0:128, 33 * (4 + j):33 * (4 + j) + VA],
                               probs[b:b + 25, co + 128:co + 256],
                               vp_sb[b:b + 25, VPC + pr * VA:VPC + (pr + 1) * VA],
                               start=False, stop=True)
                        nch = _n_chunks(t)
                        qb = 64 * (j % 2)
                        for c in range(nch):
                            nco = n_cols[(j, c)]
                            mm(pv[qb:qb + 64, 33 * (8 + j // 2):33 * (8 + j // 2) + VA],
                               probs[0:128, nco:nco + 64],
                               vslice(t, c),
                               start=(c == 0), stop=(c == nch - 1))

                    # ---- normalize: one recip + one mul per quad
                    recips = recipsp.tile([128, 16], F32)
                    nc.vector.reciprocal(recips[0:128, 0:10],
                                         pv[0:128, 32:10 * VA:VA])
                    num_v = _strided2(pv[0:128, 0:1], (VA, 10), (1, 32))
                    rec_b = _strided2(recips[0:128, 0:1], (1, 10), (0, 32))
                    dst_v = _strided2(outst[0:128, 320 * g:320 * g + 1],
                                      (32, 10), (1, 32))
                    nc.vector.tensor_mul(dst_v, num_v, rec_b)

                    # out-DMA per quad on the Pool queue, emitted at the
                    # start of the next bh iteration (or at the end)
                    def _emit_out(last=False, i=i, g=g, outst=outst):
                        # trailing outs go via SP (shorter completion sem)
                        eng = nc.sync if last else nc.gpsimd
                        eng.dma_start(
                            out=out_d[i, :, 320 * g:320 * g + 320],
                            in_=outst[:, 320 * g:320 * g + 320])
                    pending_outs.append(_emit_out)
            for fn in pending_outs:
                fn(last=True)
    nc.compile()
    return nc


_NC = None


def _get_nc():
    global _NC
    if _NC is None:
        _NC = build_nc()
    return _NC


# ---------------------------------------------------------------- entry point

def kernel(q, k, v, feats_per_t, window_len, act_size, img_feat_size):
    assert int(feats_per_t) == F and int(window_len) == W
    assert int(act_size) == 16 and int(img_feat_size) == 196
    q = np.asarray(q, np.float32)
    k = np.asarray(k, np.float32)
    v = np.asarray(v, np.float32)

    packed = _pack_all(q, k, v)
    in_maps = []
    for core in range(N_CORES):
        s = slice(BH_PER_CORE * core, BH_PER_CORE * (core + 1))
        in_maps.append({n: np.ascontiguousarray(a[s]) for n, a in packed.items()})

    nc = _get_nc()
    res = run_bass_kernel_spmd(nc, in_maps, list(range(N_CORES)))
    out = np.empty((B * H, S, D), np.float32)
    for core in range(N_CORES):
        out[BH_PER_CORE * core:BH_PER_CORE * (core + 1)] = _unpack(
            res.results[core]["out"])
    return out.reshape(B, H, S, D)


# revision 32
# speedup vs baseline: 1.0768x; 1.0768x over previous
"""Trainium2 Bass kernel for nn_EyeRobotAgent block-sparse ("eye") attention.

Shapes: q,k,v [2, 12, 3456, 32] fp32.  S = 16 time-blocks x 216 feats.
Mask structure (per query block t):
  - img queries (m in [20,216), 196 of them) see only the "core" keys:
    19 keys (m in {0..3,5..19}) of each block tau in [t-7, t] plus m4(t)
    -> at most 153 keys,
  - non-img queries (m in [0,20), 20 of them) see core keys + the 196 img
    keys of block t (joint queries additionally lose past joint keys,
    handled by a bias row).

Strategy (data-parallel: 24 (b,h) pairs over 8 cores, 3 each):
  Per block pack keys as [core 153 | img 196 | pad] = 384 (newest-first so
  invalid tail cols are contiguous; masks fold into 2 bias contraction
  rows).  Scores are computed transposed [kv, q] in per-quad (4 block)
  PSUM tiles so a single ACT exp covers ~1000-1250 columns:
    N_j: 20 non-img queries vs 2-3 full-height 128-row chunks of the pack
    A_j: 196 img queries vs core[0:128]
    C:   core[128:153] tails of the 4 blocks packed into 32-row PE
         quadrant bands (tile_position rows 32j), one shared 196-col region
    PAD: 28 dummy cols kept defined so PV lhsT "spill" reads stay legal.
  PV uses probs as the stationary operand (out[q, 33] per matmul streams
  only 33 columns); the appended ones-column of V yields the softmax
  denominator in col 32; normalize is one DVE reciprocal + one mul per
  quad, padded out-groups making every partition defined.
  exp() has no max-subtraction (scores are O(6), fp32-safe).
"""
import math
import numpy as np

import concourse.bass as bass
import concourse.mybir as mybir
import concourse.tile as tile
from concourse import bacc
from concourse.bass_utils import run_bass_kernel_spmd
from concourse.tile_rust import add_dep_helper

B, H, S, D = 2, 12, 3456, 32
F = 216            # feats_per_t
W = 8              # window_len
T = S // F         # 16 blocks
IMG_START = 20     # F - img_feat_size
PAST_SEL = np.array([0, 1, 2, 3] + list(range(5, 20)))   # 19 per block
NCORE = 153        # 8*19 + 1 (m4) candidate core keys
NIMG = 196
PACK = 384         # [core (<=153, compact) | img 196 | pad]
KAUG = D + 3       # 35 = 32 d + joint-bias + img-img bias + validity rows
VA = D + 1         # 33 = v columns + ones column
NEG = np.float32(-30000.0)
SCALE = float(1.0 / np.sqrt(np.float32(D)))
N_CORES = 8
BH_PER_CORE = (B * H) // N_CORES      # 3
NQ = 4                                # blocks per quad
QUADS = T // NQ                       # 4

F32 = mybir.dt.float32
BF16 = mybir.dt.float16      # half precision: matmul rate 1 cyc/row
NP_BF16 = np.float16


def _nvalid_core(t):
    return 20 + 19 * min(t, 7)


def _n_chunks(t):
    return math.ceil((_nvalid_core(t) + NIMG) / 128)


def _pack_rows(t):
    """Compact key packing for block t: [19(t), m4(t), 19(t-1), ..,
    19(t-min(t,7)), img(t) 196, pad].  -1 marks invalid (trailing only)."""
    rows = list(F * t + PAST_SEL) + [F * t + 4]
    for s in range(1, min(t, 7) + 1):
        rows += list(F * (t - s) + PAST_SEL)
    rows += list(range(F * t + IMG_START, F * t + F))
    rows += [-1] * (PACK - len(rows))
    return np.array(rows)


def _quad_layout(g):
    """Column layout of the per-quad score tile, bank-aware (matmul outputs
    must not cross 512-col PSUM bank boundaries): bank0 = [A0 A1 N x6],
    bank1 = [A2 A3 N-rest pad?], bank2 = [C... pad].  C regions are per
    block PAIR (bands rows 0:32 / 32:64 by t%2, rows 64:128 dummy-filled).
    Exp'd cols [0:ncols] are gap-free; 60 pad cols keep PV spill reads
    defined."""
    ts = list(range(NQ * g, NQ * g + NQ))
    chunks = [(j, c) for j, t in enumerate(ts) for c in range(_n_chunks(t))]
    a_off = [0, 196, 512, 708]
    n_cols = {}
    col = 392
    for jc in chunks[:6]:
        n_cols[jc] = col
        col += 20
    assert col <= 512
    col = 904
    for jc in chunks[6:]:
        n_cols[jc] = col
        col += 20
    pads = []
    if col < 1024:
        pads.append((col, 1024 - col))
    c_off = {}
    cbase = 1024
    for pl in range(NQ // 2):           # local pair index
        if any(_nvalid_core(t) > 128 for t in ts[2 * pl:2 * pl + 2]):
            c_off[pl] = cbase
            cbase += NIMG
    pads.append((cbase, 60))
    return ts, n_cols, a_off, c_off, pads, cbase + 60


# ---------------------------------------------------------------- host packing

def _pack_all(q, k, v):
    nbh = B * H
    qf = q.reshape(nbh, S, D)
    kf = k.reshape(nbh, S, D)
    vf = v.reshape(nbh, S, D)
    qm = np.arange(S) % F

    # qt/kpt live in two partition bands (rows 0:35 = blocks 0..7, rows
    # 64:99 = blocks 8..15): v1-model DMA cost is per-partition bytes, so
    # spreading over 2x partitions halves the transfer time.
    qtf = np.zeros((nbh, KAUG, S), np.float32)
    qtf[:, :D] = qf.transpose(0, 2, 1)
    qtf[:, 32] = (qm >= 4) & (qm < IMG_START)      # is_joint(q)
    qtf[:, 33] = qm >= IMG_START                   # is_img(q)
    qtf[:, 34] = 1.0
    qt = np.zeros((nbh, 99, S // 2), np.float32)
    qt[:, 0:KAUG] = qtf[:, :, 0:S // 2]
    qt[:, 64:64 + KAUG] = qtf[:, :, S // 2:]

    kpt = np.zeros((nbh, 99, T // 2, PACK), np.float32)
    vp = np.zeros((nbh, 128, T, 3, VA), np.float32)
    vpc = np.zeros((nbh, 128, T // 2, VA), np.float32)
    for t in range(T):
        rows = _pack_rows(t)
        valid = rows >= 0
        safe = np.where(valid, rows, 0)
        kb, tl = 64 * (t // 8), t % 8
        kpt[:, kb:kb + D, tl] = np.where(
            valid[None, None], kf[:, safe].transpose(0, 2, 1), 0.0)
        # joint-past bias: past sets s=1..min(t,7) at cols 20+19(s-1),
        # joint keys at positions 4..18 within each set
        jbias = np.zeros(PACK, np.float32)
        for s in range(1, min(t, 7) + 1):
            base = 20 + 19 * (s - 1)
            jbias[base + 4: base + 19] = NEG
        kpt[:, kb + 32, tl] = jbias
        kpt[:, kb + 33, tl] = np.where(valid & (rows % F >= IMG_START), NEG, 0.0)
        kpt[:, kb + 34, tl] = np.where(valid, 0.0, NEG)
        vblk = np.where(valid[None, :, None], vf[:, safe], 0.0)   # [nbh,384,32]
        vp[:, :, t, :, :D] = vblk.reshape(nbh, 3, 128, D).transpose(0, 2, 1, 3)
        # ones column: eps (not 0) on invalid rows keeps every PV spill-row
        # denominator strictly positive (invalid probs are exactly 0, so
        # valid outputs are unaffected).
        ones = np.where(valid, 1.0, 6e-5)
        vp[:, :, t, :, 32] = ones.reshape(3, 128).T[None]
        pr, b = t // 2, 32 * (t % 2)
        # C tail: only core positions 128..nvalid_core-1 (img keys that
        # fall in [128:153] of the compact pack must contribute zero)
        ncv = _nvalid_core(t)
        tail = safe[128:NCORE]
        tval = np.arange(128, NCORE) < ncv
        vpc[:, b:b + 25, pr, :D] = np.where(tval[None, :, None], vf[:, tail], 0.0)
        vpc[:, b:b + 25, pr, 32] = np.where(tval, 1.0, 0.0)
    # fold vpc after vp in one tensor (one DMA)
    vcomb = np.concatenate(
        [vp.reshape(nbh, 128, T * 3 * VA), vpc.reshape(nbh, 128, T // 2 * VA)],
        axis=2)
    return {"qt": np.ascontiguousarray(qt.astype(NP_BF16)),
            "kpt": np.ascontiguousarray(kpt.astype(NP_BF16)),
            "vp": np.ascontiguousarray(vcomb.astype(NP_BF16))}


def _unpack(arr):
    """arr [n, 128, QUADS*320] staging -> [n, S, D].  Per quad g, 10 groups
    of 32 cols: j=0..3 img q 20..147 (rows 0:128); 4..7 img q 148..215
    (rows 0:68); 8..9 non-img q 0..19 of blocks 2p (rows 0:20) and 2p+1
    (rows 64:84)."""
    n = arr.shape[0]
    arr = arr.astype(np.float32)
    r = arr.reshape(n, 128, QUADS, 10, 32).transpose(0, 2, 3, 1, 4)
    out = np.empty((n, QUADS, NQ, F, D), np.float32)
    for j in range(NQ):
        out[:, :, j, IMG_START:148] = r[:, :, j, 0:128]
        out[:, :, j, 148:] = r[:, :, 4 + j, 0:68]
        out[:, :, j, :IMG_START] = r[:, :, 8 + j // 2, 64 * (j % 2):64 * (j % 2) + 20]
    return out.reshape(n, S, D)


# ---------------------------------------------------------------- bass kernel

def build_nc(n_bh=BH_PER_CORE):
    nc = bacc.Bacc(None, target_bir_lowering=False, debug=False)
    qt_d = nc.declare_dram_parameter("qt", [n_bh, 99, S // 2], BF16, isOutput=False)
    kpt_d = nc.declare_dram_parameter("kpt", [n_bh, 99, T // 2, PACK], BF16, isOutput=False)
    vp_d = nc.declare_dram_parameter("vp", [n_bh, 128, (T * 3 + T // 2) * VA], BF16, isOutput=False)
    # out mirrors the SBUF staging tile exactly (fp16); host unpacks.
    out_d = nc.declare_dram_parameter("out", [n_bh, 128, QUADS * 320], BF16, isOutput=True)

    def _strided2(ap, d1, d2):
        return bass.AP(tensor=ap.tensor, offset=ap.offset,
                       ap=[list(ap.ap[0]), list(d1), list(d2)])

    with tile.TileContext(nc) as tc:
        with (
            tc.tile_pool(name="singles", bufs=1) as singles,
            tc.tile_pool(name="qtp", bufs=3) as qtp,
            tc.tile_pool(name="kptp", bufs=3) as kptp,
            tc.tile_pool(name="vpp", bufs=3) as vpp,
            tc.tile_pool(name="probsp", bufs=3) as probsp,
            tc.tile_pool(name="recipsp", bufs=3) as recipsp,
            tc.tile_pool(name="outsbp", bufs=3) as outsbp,
            tc.tile_pool(name="scoresp", bufs=2, space="PSUM") as scoresp,
            tc.tile_pool(name="pvp", bufs=2, space="PSUM") as pvp,
        ):
            zero = singles.tile([1, 128], BF16)
            nc.vector.memset(zero[:], 0.0)

            pending_outs = []
            for i in range(n_bh):
                qt_sb = qtp.tile([99, S // 2], BF16)
                kpt_sb = kptp.tile([99, T // 2, PACK], BF16)
                vp_sb = vpp.tile([128, (T * 3 + T // 2) * VA], BF16)
                # halves so quad 0 can start after ~half the input latency;
                # kpt.h1 goes on the Pool queue, parallel with qt.h1 on SP.
                # Prior-bh out-DMAs are emitted AFTER this bh's inputs so
                # they never head-of-line-block the input stream.
                VH = T * 3 * VA // 2
                nc.gpsimd.dma_start(out=kpt_sb[:, 0:T // 4, :],
                                    in_=kpt_d[i, :, 0:T // 4, :])
                nc.sync.dma_start(out=qt_sb[:, 0:S // 4],
                                  in_=qt_d[i, :, 0:S // 4])
                nc.sync.dma_start(out=qt_sb[:, S // 4:],
                                  in_=qt_d[i, :, S // 4:])
                nc.sync.dma_start(out=kpt_sb[:, T // 4:, :],
                                  in_=kpt_d[i, :, T // 4:, :])
                nc.gpsimd.dma_start(out=vp_sb[:, 0:VH], in_=vp_d[i, :, 0:VH])
                nc.gpsimd.dma_start(out=vp_sb[:, VH:], in_=vp_d[i, :, VH:])
                for fn in pending_outs:
                    fn()
                pending_outs = []
                outst = outsbp.tile([128, QUADS * 320], BF16)

                VPC = T * 3 * VA          # vpc offset inside vp_sb

                def kslice(t, lo, hi):
                    return kpt_sb[64 * (t // 8):64 * (t // 8) + KAUG,
                                  t % 8, lo:hi]

                def qslice(t, lo, hi):
                    base = F * t - (S // 2) * (t // 8)
                    return qt_sb[64 * (t // 8):64 * (t // 8) + KAUG,
                                 base + lo:base + hi]

                def vslice(t, c):
                    return vp_sb[:, (3 * t + c) * VA:(3 * t + c + 1) * VA]

                for g in range(QUADS):
                    ts, n_cols, a_off, c_off, pads, ncols = _quad_layout(g)

                    # ---- QK^T (transposed scores [kv, q]); masks via the 2
                    # bias rows; all mms chained (same-bank group ordering).
                    scores = scoresp.tile([128, 1536], F32)
                    prev = None

                    def mm(out, lhsT, rhs, start=True, stop=True):
                        nonlocal prev
                        m = nc.tensor.matmul(out, lhsT=lhsT, rhs=rhs,
                                             start=start, stop=stop)
                        if prev is not None:
                            add_dep_helper(m.ins, prev.ins, sync=False,
                                           reason="bank group order")
                        prev = m
                        return m

                    for j, t in enumerate(ts):
                        for c in range(_n_chunks(t)):
                            nco = n_cols[(j, c)]
                            mm(scores[0:128, nco:nco + 20],
                               kslice(t, 128 * c, 128 * c + 128),
                               qslice(t, 0, IMG_START))
                    for j, t in enumerate(ts):
                        mm(scores[0:128, a_off[j]:a_off[j] + NIMG],
                           kslice(t, 0, 128),
                           qslice(t, IMG_START, F))
                    for pl, co in c_off.items():
                        for t in ts[2 * pl:2 * pl + 2]:
                            b = 32 * (t % 2)
                            if _nvalid_core(t) > 128:
                                mm(scores[b:b + 32, co:co + NIMG],
                                   kslice(t, 128, 160),
                                   qslice(t, IMG_START, F))
                            else:      # dummy fill: zero scores -> probs 1
                                mm(scores[b:b + 32, co:co + NIMG],
                                   zero[0:1, 0:32], qt_sb[0:1, 0:NIMG])
                        mm(scores[64:128, co:co + NIMG],
                           zero[0:1, 0:64], qt_sb[0:1, 0:NIMG])


                    # ---- probs = exp(scale * scores), one ACT op per
                    # quad; trailing pad cols (PV spill targets) don't need
                    # exp -- Pool memsets them to 1.0 directly.
                    probs = probsp.tile([128, 1536], BF16)
                    pad0 = pads[0][0]
                    nc.scalar.activation(probs[0:128, 0:pad0],
                                         scores[0:128, 0:pad0],
                                         mybir.ActivationFunctionType.Exp,
                                         scale=SCALE)
                    nc.gpsimd.memset(probs[0:128, pad0:ncols], 1.0)

                    # ---- PV: probs stationary, out[q, va] (33-col streams).
                    # 10 out groups: col 33j img q 0..127; col 33(4+j) img q
                    # 128..195 (68 valid + 60 spill); col 33(8+p) non-img of
                    # blocks 2p (rows 0:64) / 2p+1 (rows 64:128), 20 valid
                    # + 44 spill each.  Spill keeps all partitions defined
                    # so one recip + one mul normalizes everything.
                    pv = pvp.tile([128, 10 * VA], F32)
                    prev = None
                    for j, t in enumerate(ts):
                        has_c = _nvalid_core(t) > 128
                        co = c_off.get(j // 2)
                        b = 32 * (t % 2)
                        pr = t // 2
                        mm(pv[0:128, 33 * j:33 * j + VA],
                           probs[0:128, a_off[j]:a_off[j] + 128],
                           vslice(t, 0), start=True, stop=not has_c)
                        if has_c:
                            mm(pv[0:128, 33 * j:33 * j + VA],
                               probs[b:b + 25, co:co + 128],
                               vp_sb[b:b + 25, VPC + pr * VA:VPC + (pr + 1) * VA],
                               start=False, stop=True)
                        mm(pv[0:128, 33 * (4 + j):33 * (4 + j) + VA],
                           probs[0:128, a_off[j] + 128:a_off[j] + 256],
                           vslice(t, 0), start=True, stop=not has_c)
                        if has_c:
                            mm(pv[0:128, 33 * (4 + j):33 * (4 + j) + VA],
                               probs[b:b + 25, co + 128:co + 256],
                               vp_sb[b:b + 25, VPC + pr * VA:VPC + (pr + 1) * VA],
                               start=False, stop=True)
                        nch = _n_chunks(t)
                        qb = 64 * (j % 2)
                        for c in range(nch):
                            nco = n_cols[(j, c)]
                            mm(pv[qb:qb + 64, 33 * (8 + j // 2):33 * (8 + j // 2) + VA],
                               probs[0:128, nco:nco + 64],
                               vslice(t, c),
                               start=(c == 0), stop=(c == nch - 1))

                    # ---- normalize: one recip + one mul per quad
                    recips = recipsp.tile([128, 16], F32)
                    nc.vector.reciprocal(recips[0:128, 0:10],
                                         pv[0:128, 32:10 * VA:VA])
                    num_v = _strided2(pv[0:128, 0:1], (VA, 10), (1, 32))
                    rec_b = _strided2(recips[0:128, 0:1], (1, 10), (0, 32))
                    dst_v = _strided2(outst[0:128, 320 * g:320 * g + 1],
                                      (32, 10), (1, 32))
                    nc.vector.tensor_mul(dst_v, num_v, rec_b)

                    # out-DMA per quad on the Pool queue, emitted at the
                    # start of the next bh iteration (or at the end)
                    def _emit_out(last=False, i=i, g=g, outst=outst):
                        # trailing outs go via SP (shorter completion sem)
                        eng = nc.sync if last else nc.gpsimd
                        eng.dma_start(
                            out=out_d[i, :, 320 * g:320 * g + 320],
                            in_=outst[:, 320 * g:320 * g + 320])
                    pending_outs.append(_emit_out)
            for fn in pending_outs:
                fn(last=True)
    nc.compile()
    return nc


_NC = None


def _get_nc():
    global _NC
    if _NC is None:
        _NC = build_nc()
    return _NC


# ---------------------------------------------------------------- entry point

def kernel(q, k, v, feats_per_t, window_len, act_size, img_feat_size):
    assert int(feats_per_t) == F and int(window_len) == W
    assert int(act_size) == 16 and int(img_feat_size) == 196
    q = np.asarray(q, np.float32)
    k = np.asarray(k, np.float32)
    v = np.asarray(v, np.float32)

    packed = _pack_all(q, k, v)
    in_maps = []
    for core in range(N_CORES):
        s = slice(BH_PER_CORE * core, BH_PER_CORE * (core + 1))
        in_maps.append({n: np.ascontiguousarray(a[s]) for n, a in packed.items()})

    nc = _get_nc()
    res = run_bass_kernel_spmd(nc, in_maps, list(range(N_CORES)))
    out = np.empty((B * H, S, D), np.float32)
    for core in range(N_CORES):
        out[BH_PER_CORE * core:BH_PER_CORE * (core + 1)] = _unpack(
            res.results[core]["out"])
    return out.reshape(B, H, S, D)


# revision 33
# speedup vs baseline: 1.0800x; 1.0029x over previous
"""Trainium2 Bass kernel for nn_EyeRobotAgent block-sparse ("eye") attention.

Shapes: q,k,v [2, 12, 3456, 32] fp32.  S = 16 time-blocks x 216 feats.
Mask structure (per query block t):
  - img queries (m in [20,216), 196 of them) see only the "core" keys:
    19 keys (m in {0..3,5..19}) of each block tau in [t-7, t] plus m4(t)
    -> at most 153 keys,
  - non-img queries (m in [0,20), 20 of them) see core keys + the 196 img
    keys of block t (joint queries additionally lose past joint keys,
    handled by a bias row).

Strategy (data-parallel: 24 (b,h) pairs over 8 cores, 3 each):
  Per block pack keys as [core 153 | img 196 | pad] = 384 (newest-first so
  invalid tail cols are contiguous; masks fold into 2 bias contraction
  rows).  Scores are computed transposed [kv, q] in per-quad (4 block)
  PSUM tiles so a single ACT exp covers ~1000-1250 columns:
    N_j: 20 non-img queries vs 2-3 full-height 128-row chunks of the pack
    A_j: 196 img queries vs core[0:128]
    C:   core[128:153] tails of the 4 blocks packed into 32-row PE
         quadrant bands (tile_position rows 32j), one shared 196-col region
    PAD: 28 dummy cols kept defined so PV lhsT "spill" reads stay legal.
  PV uses probs as the stationary operand (out[q, 33] per matmul streams
  only 33 columns); the appended ones-column of V yields the softmax
  denominator in col 32; normalize is one DVE reciprocal + one mul per
  quad, padded out-groups making every partition defined.
  exp() has no max-subtraction (scores are O(6), fp32-safe).
"""
import math
import numpy as np

import concourse.bass as bass
import concourse.mybir as mybir
import concourse.tile as tile
from concourse import bacc
from concourse.bass_utils import run_bass_kernel_spmd
from concourse.tile_rust import add_dep_helper

B, H, S, D = 2, 12, 3456, 32
F = 216            # feats_per_t
W = 8              # window_len
T = S // F         # 16 blocks
IMG_START = 20     # F - img_feat_size
PAST_SEL = np.array([0, 1, 2, 3] + list(range(5, 20)))   # 19 per block
NCORE = 153        # 8*19 + 1 (m4) candidate core keys
NIMG = 196
PACK = 384         # [core (<=153, compact) | img 196 | pad]
KAUG = D + 3       # 35 = 32 d + joint-bias + img-img bias + validity rows
VA = D + 1         # 33 = v columns + ones column
NEG = np.float32(-30000.0)
SCALE = float(1.0 / np.sqrt(np.float32(D)))
N_CORES = 8
BH_PER_CORE = (B * H) // N_CORES      # 3
NQ = 4                                # blocks per quad
QUADS = T // NQ                       # 4

F32 = mybir.dt.float32
BF16 = mybir.dt.float16      # half precision: matmul rate 1 cyc/row
NP_BF16 = np.float16


def _nvalid_core(t):
    return 20 + 19 * min(t, 7)


def _n_chunks(t):
    return math.ceil((_nvalid_core(t) + NIMG) / 128)


def _pack_rows(t):
    """Compact key packing for block t: [19(t), m4(t), 19(t-1), ..,
    19(t-min(t,7)), img(t) 196, pad].  -1 marks invalid (trailing only)."""
    rows = list(F * t + PAST_SEL) + [F * t + 4]
    for s in range(1, min(t, 7) + 1):
        rows += list(F * (t - s) + PAST_SEL)
    rows += list(range(F * t + IMG_START, F * t + F))
    rows += [-1] * (PACK - len(rows))
    return np.array(rows)


def _quad_layout(g):
    """Column layout of the per-quad score tile, bank-aware (matmul outputs
    must not cross 512-col PSUM bank boundaries): bank0 = [A0 A1 N x6],
    bank1 = [A2 A3 N-rest pad?], bank2 = [C... pad].  C regions are per
    block PAIR (bands rows 0:32 / 32:64 by t%2, rows 64:128 dummy-filled).
    Exp'd cols [0:ncols] are gap-free; 60 pad cols keep PV spill reads
    defined."""
    ts = list(range(NQ * g, NQ * g + NQ))
    chunks = [(j, c) for j, t in enumerate(ts) for c in range(_n_chunks(t))]
    a_off = [0, 196, 512, 708]
    n_cols = {}
    col = 392
    for jc in chunks[:6]:
        n_cols[jc] = col
        col += 20
    assert col <= 512
    col = 904
    for jc in chunks[6:]:
        n_cols[jc] = col
        col += 20
    pads = []
    if col < 1024:
        pads.append((col, 1024 - col))
    c_off = {}
    cbase = 1024
    for pl in range(NQ // 2):           # local pair index
        if any(_nvalid_core(t) > 128 for t in ts[2 * pl:2 * pl + 2]):
            c_off[pl] = cbase
            cbase += NIMG
    pads.append((cbase, 60))
    return ts, n_cols, a_off, c_off, pads, cbase + 60


# ---------------------------------------------------------------- host packing

def _pack_all(q, k, v):
    nbh = B * H
    qf = q.reshape(nbh, S, D)
    kf = k.reshape(nbh, S, D)
    vf = v.reshape(nbh, S, D)
    qm = np.arange(S) % F

    # qt/kpt live in two partition bands (rows 0:35 = blocks 0..7, rows
    # 64:99 = blocks 8..15): v1-model DMA cost is per-partition bytes, so
    # spreading over 2x partitions halves the transfer time.
    qtf = np.zeros((nbh, KAUG, S), np.float32)
    qtf[:, :D] = qf.transpose(0, 2, 1)
    qtf[:, 32] = (qm >= 4) & (qm < IMG_START)      # is_joint(q)
    qtf[:, 33] = qm >= IMG_START                   # is_img(q)
    qtf[:, 34] = 1.0
    qt = np.zeros((nbh, 99, S // 2), np.float32)
    qt[:, 0:KAUG] = qtf[:, :, 0:S // 2]
    qt[:, 64:64 + KAUG] = qtf[:, :, S // 2:]

    kpt = np.zeros((nbh, 99, T // 2, PACK), np.float32)
    vp = np.zeros((nbh, 128, T, 3, VA), np.float32)
    vpc = np.zeros((nbh, 128, T // 2, VA), np.float32)
    for t in range(T):
        rows = _pack_rows(t)
        valid = rows >= 0
        safe = np.where(valid, rows, 0)
        kb, tl = 64 * (t // 8), t % 8
        kpt[:, kb:kb + D, tl] = np.where(
            valid[None, None], kf[:, safe].transpose(0, 2, 1), 0.0)
        # joint-past bias: past sets s=1..min(t,7) at cols 20+19(s-1),
        # joint keys at positions 4..18 within each set
        jbias = np.zeros(PACK, np.float32)
        for s in range(1, min(t, 7) + 1):
            base = 20 + 19 * (s - 1)
            jbias[base + 4: base + 19] = NEG
        kpt[:, kb + 32, tl] = jbias
        kpt[:, kb + 33, tl] = np.where(valid & (rows % F >= IMG_START), NEG, 0.0)
        kpt[:, kb + 34, tl] = np.where(valid, 0.0, NEG)
        vblk = np.where(valid[None, :, None], vf[:, safe], 0.0)   # [nbh,384,32]
        vp[:, :, t, :, :D] = vblk.reshape(nbh, 3, 128, D).transpose(0, 2, 1, 3)
        # ones column: eps (not 0) on invalid rows keeps every PV spill-row
        # denominator strictly positive (invalid probs are exactly 0, so
        # valid outputs are unaffected).
        ones = np.where(valid, 1.0, 6e-5)
        vp[:, :, t, :, 32] = ones.reshape(3, 128).T[None]
        pr, b = t // 2, 32 * (t % 2)
        # C tail: only core positions 128..nvalid_core-1 (img keys that
        # fall in [128:153] of the compact pack must contribute zero)
        ncv = _nvalid_core(t)
        tail = safe[128:NCORE]
        tval = np.arange(128, NCORE) < ncv
        vpc[:, b:b + 25, pr, :D] = np.where(tval[None, :, None], vf[:, tail], 0.0)
        vpc[:, b:b + 25, pr, 32] = np.where(tval, 1.0, 0.0)
    # fold vpc after vp in one tensor (one DMA)
    vcomb = np.concatenate(
        [vp.reshape(nbh, 128, T * 3 * VA), vpc.reshape(nbh, 128, T // 2 * VA)],
        axis=2)
    return {"qt": np.ascontiguousarray(qt.astype(NP_BF16)),
            "kpt": np.ascontiguousarray(kpt.astype(NP_BF16)),
            "vp": np.ascontiguousarray(vcomb.astype(NP_BF16))}


def _unpack(arr):
    """arr [n, 128, QUADS*320] staging -> [n, S, D].  Per quad g, 10 groups
    of 32 cols: j=0..3 img q 20..147 (rows 0:128); 4..7 img q 148..215
    (rows 0:68); 8..9 non-img q 0..19 of blocks 2p (rows 0:20) and 2p+1
    (rows 64:84)."""
    n = arr.shape[0]
    arr = arr.astype(np.float32)
    r = arr.reshape(n, 128, QUADS, 10, 32).transpose(0, 2, 3, 1, 4)
    out = np.empty((n, QUADS, NQ, F, D), np.float32)
    for j in range(NQ):
        out[:, :, j, IMG_START:148] = r[:, :, j, 0:128]
        out[:, :, j, 148:] = r[:, :, 4 + j, 0:68]
        out[:, :, j, :IMG_START] = r[:, :, 8 + j // 2, 64 * (j % 2):64 * (j % 2) + 20]
    return out.reshape(n, S, D)


# ---------------------------------------------------------------- bass kernel

def build_nc(n_bh=BH_PER_CORE):
    nc = bacc.Bacc(None, target_bir_lowering=False, debug=False)
    qt_d = nc.declare_dram_parameter("qt", [n_bh, 99, S // 2], BF16, isOutput=False)
    kpt_d = nc.declare_dram_parameter("kpt", [n_bh, 99, T // 2, PACK], BF16, isOutput=False)
    vp_d = nc.declare_dram_parameter("vp", [n_bh, 128, (T * 3 + T // 2) * VA], BF16, isOutput=False)
    # out mirrors the SBUF staging tile exactly (fp16); host unpacks.
    out_d = nc.declare_dram_parameter("out", [n_bh, 128, QUADS * 320], BF16, isOutput=True)

    def _strided2(ap, d1, d2):
        return bass.AP(tensor=ap.tensor, offset=ap.offset,
                       ap=[list(ap.ap[0]), list(d1), list(d2)])

    with tile.TileContext(nc) as tc:
        with (
            tc.tile_pool(name="singles", bufs=1) as singles,
            tc.tile_pool(name="qtp", bufs=3) as qtp,
            tc.tile_pool(name="kptp", bufs=3) as kptp,
            tc.tile_pool(name="vpp", bufs=3) as vpp,
            tc.tile_pool(name="probsp", bufs=3) as probsp,
            tc.tile_pool(name="recipsp", bufs=3) as recipsp,
            tc.tile_pool(name="outsbp", bufs=3) as outsbp,
            tc.tile_pool(name="scoresp", bufs=2, space="PSUM") as scoresp,
            tc.tile_pool(name="pvp", bufs=2, space="PSUM") as pvp,
        ):
            zero = singles.tile([1, 128], BF16)
            nc.vector.memset(zero[:], 0.0)

            pending_outs = []
            for i in range(n_bh):
                qt_sb = qtp.tile([99, S // 2], BF16)
                kpt_sb = kptp.tile([99, T // 2, PACK], BF16)
                vp_sb = vpp.tile([128, (T * 3 + T // 2) * VA], BF16)
                # halves so quad 0 can start after ~half the input latency;
                # kpt.h1 goes on the Pool queue, parallel with qt.h1 on SP.
                # Prior-bh out-DMAs are emitted AFTER this bh's inputs so
                # they never head-of-line-block the input stream.
                VH = T * 3 * VA // 2
                if i == 0:
                    # SP's DMA completion latency is 167ns shorter than the
                    # Pool/SWDGE path; the first kpt piece gates quad 0
                    nc.sync.dma_start(out=kpt_sb[:, 0:T // 4, :],
                                      in_=kpt_d[i, :, 0:T // 4, :])
                    nc.gpsimd.dma_start(out=qt_sb[:, 0:S // 4],
                                        in_=qt_d[i, :, 0:S // 4])
                else:
                    nc.gpsimd.dma_start(out=kpt_sb[:, 0:T // 4, :],
                                        in_=kpt_d[i, :, 0:T // 4, :])
                    nc.sync.dma_start(out=qt_sb[:, 0:S // 4],
                                      in_=qt_d[i, :, 0:S // 4])
                nc.sync.dma_start(out=qt_sb[:, S // 4:],
                                  in_=qt_d[i, :, S // 4:])
                nc.sync.dma_start(out=kpt_sb[:, T // 4:, :],
                                  in_=kpt_d[i, :, T // 4:, :])
                nc.gpsimd.dma_start(out=vp_sb[:, 0:VH], in_=vp_d[i, :, 0:VH])
                nc.gpsimd.dma_start(out=vp_sb[:, VH:], in_=vp_d[i, :, VH:])
                for fn in pending_outs:
                    fn()
                pending_outs = []
                outst = outsbp.tile([128, QUADS * 320], BF16)

                VPC = T * 3 * VA          # vpc offset inside vp_sb

                def kslice(t, lo, hi):
                    return kpt_sb[64 * (t // 8):64 * (t // 8) + KAUG,
                                  t % 8, lo:hi]

                def qslice(t, lo, hi):
                    base = F * t - (S // 2) * (t // 8)
                    return qt_sb[64 * (t // 8):64 * (t // 8) + KAUG,
                                 base + lo:base + hi]

                def vslice(t, c):
                    return vp_sb[:, (3 * t + c) * VA:(3 * t + c + 1) * VA]

                for g in range(QUADS):
                    ts, n_cols, a_off, c_off, pads, ncols = _quad_layout(g)

                    # ---- QK^T (transposed scores [kv, q]); masks via the 2
                    # bias rows; all mms chained (same-bank group ordering).
                    scores = scoresp.tile([128, 1536], F32)
                    prev = None

                    def mm(out, lhsT, rhs, start=True, stop=True):
                        nonlocal prev
                        m = nc.tensor.matmul(out, lhsT=lhsT, rhs=rhs,
                                             start=start, stop=stop)
                        if prev is not None:
                            add_dep_helper(m.ins, prev.ins, sync=False,
                                           reason="bank group order")
                        prev = m
                        return m

                    for j, t in enumerate(ts):
                        for c in range(_n_chunks(t)):
                            nco = n_cols[(j, c)]
                            mm(scores[0:128, nco:nco + 20],
                               kslice(t, 128 * c, 128 * c + 128),
                               qslice(t, 0, IMG_START))
                    for j, t in enumerate(ts):
                        mm(scores[0:128, a_off[j]:a_off[j] + NIMG],
                           kslice(t, 0, 128),
                           qslice(t, IMG_START, F))
                    for pl, co in c_off.items():
                        for t in ts[2 * pl:2 * pl + 2]:
                            b = 32 * (t % 2)
                            if _nvalid_core(t) > 128:
                                mm(scores[b:b + 32, co:co + NIMG],
                                   kslice(t, 128, 160),
                                   qslice(t, IMG_START, F))
                            else:      # dummy fill: zero scores -> probs 1
                                mm(scores[b:b + 32, co:co + NIMG],
                                   zero[0:1, 0:32], qt_sb[0:1, 0:NIMG])
                        mm(scores[64:128, co:co + NIMG],
                           zero[0:1, 0:64], qt_sb[0:1, 0:NIMG])


                    # ---- probs = exp(scale * scores), one ACT op per
                    # quad; trailing pad cols (PV spill targets) don't need
                    # exp -- Pool memsets them to 1.0 directly.
                    probs = probsp.tile([128, 1536], BF16)
                    pad0 = pads[0][0]
                    nc.scalar.activation(probs[0:128, 0:pad0],
                                         scores[0:128, 0:pad0],
                                         mybir.ActivationFunctionType.Exp,
                                         scale=SCALE)
                    nc.gpsimd.memset(probs[0:128, pad0:ncols], 1.0)

                    # ---- PV: probs stationary, out[q, va] (33-col streams).
                    # 10 out groups: col 33j img q 0..127; col 33(4+j) img q
                    # 128..195 (68 valid + 60 spill); col 33(8+p) non-img of
                    # blocks 2p (rows 0:64) / 2p+1 (rows 64:128), 20 valid
                    # + 44 spill each.  Spill keeps all partitions defined
                    # so one recip + one mul normalizes everything.
                    pv = pvp.tile([128, 10 * VA], F32)
                    prev = None
                    for j, t in enumerate(ts):
                        has_c = _nvalid_core(t) > 128
                        co = c_off.get(j // 2)
                        b = 32 * (t % 2)
                        pr = t // 2
                        mm(pv[0:128, 33 * j:33 * j + VA],
                           probs[0:128, a_off[j]:a_off[j] + 128],
                           vslice(t, 0), start=True, stop=not has_c)
                        if has_c:
                            mm(pv[0:128, 33 * j:33 * j + VA],
                               probs[b:b + 25, co:co + 128],
                               vp_sb[b:b + 25, VPC + pr * VA:VPC + (pr + 1) * VA],
                               start=False, stop=True)
                        mm(pv[0:128, 33 * (4 + j):33 * (4 + j) + VA],
                           probs[0:128, a_off[j] + 128:a_off[j] + 256],
                           vslice(t, 0), start=True, stop=not has_c)
                        if has_c:
                            mm(pv[0:128, 33 * (4 + j):33 * (4 + j) + VA],
                               probs[b:b + 25, co + 128:co + 256],
                               vp_sb[b:b + 25, VPC + pr * VA:VPC + (pr + 1) * VA],
                               start=False, stop=True)
                        nch = _n_chunks(t)
                        qb = 64 * (j % 2)
                        for c in range(nch):
                            nco = n_cols[(j, c)]
                            mm(pv[qb:qb + 64, 33 * (8 + j // 2):33 * (8 + j // 2) + VA],
                               probs[0:128, nco:nco + 64],
                               vslice(t, c),
                               start=(c == 0), stop=(c == nch - 1))

                    # ---- normalize: one recip + one mul per quad
                    recips = recipsp.tile([128, 16], F32)
                    nc.vector.reciprocal(recips[0:128, 0:10],
                                         pv[0:128, 32:10 * VA:VA])
                    num_v = _strided2(pv[0:128, 0:1], (VA, 10), (1, 32))
                    rec_b = _strided2(recips[0:128, 0:1], (1, 10), (0, 32))
                    dst_v = _strided2(outst[0:128, 320 * g:320 * g + 1],
                                      (32, 10), (1, 32))
                    nc.vector.tensor_mul(dst_v, num_v, rec_b)

                    # out-DMA per quad on the Pool queue, emitted at the
                    # start of the next bh iteration (or at the end)
                    def _emit_out(last=False, i=i, g=g, outst=outst):
                        # trailing outs go via SP (shorter completion sem)
                        eng = nc.sync if last else nc.gpsimd
                        eng.dma_start(
                            out=out_d[i, :, 320 * g:320 * g + 320],
                            in_=outst[:, 320 * g:320 * g + 320])
                    pending_outs.append(_emit_out)
            for fn in pending_outs:
                fn(last=True)
    nc.compile()
    return nc


_NC = None


def _get_nc():
    global _NC
    if _NC is None:
        _NC = build_nc()
    return _NC


# ---------------------------------------------------------------- entry point

def kernel(q, k, v, feats_per_t, window_len, act_size, img_feat_size):
    assert int(feats_per_t) == F and int(window_len) == W
    assert int(act_size) == 16 and int(img_feat_size) == 196
    q = np.asarray(q, np.float32)
    k = np.asarray(k, np.float32)
    v = np.asarray(v, np.float32)

    packed = _pack_all(q, k, v)
    in_maps = []
    for core in range(N_CORES):
        s = slice(BH_PER_CORE * core, BH_PER_CORE * (core + 1))
        in_maps.append({n: np.ascontiguousarray(a[s]) for n, a in packed.items()})

    nc = _get_nc()
    res = run_bass_kernel_spmd(nc, in_maps, list(range(N_CORES)))
    out = np.empty((B * H, S, D), np.float32)
    for core in range(N_CORES):
        out[BH_PER_CORE * core:BH_PER_CORE * (core + 1)] = _unpack(
            res.results[core]["out"])
    return out.reshape(B, H, S, D)
